# revision 33
# baseline (speedup 1.0000x reference)
"""Arctic expert-choice MoE router on 8 Trainium2 NeuronCores (Bass/Tile).

Problem: x [16384, 4096] f32, W_gate [128, 4096] f32.
  logits = x @ W_gate.T                      [T=16384, E=128]
  expert_indices = top_k(logits.T, 160)      [E, 160]  (per-expert top tokens)
  dispatch_mask[t, e] = 1.0 for selected     [T, E]
  load_balancing_loss = mean(load * log(load/mean_load)) = 160*log(1.0)

Sharding: tokens split across 8 cores (2048 each); W replicated.

Numerics: the matmul runs as 4 fp16 passes (x and W each split into
fp16 hi+lo pairs on the host; products are exact in fp32 PSUM), which
matches fp32 logits to ~1e-6 — enough to reproduce the reference's
top-k ordering exactly (measured min adjacent-gap ~7.5e-7, typical
2.8e-3). fp16 operands let the PE run at 1 cycle/row (vs 4 for fp32)
and let the DMA xbar do the x transpose (2-byte dtypes only).

Per-core device program (SPMD), pipelined over two 1024-token halves:
  1. For each half: 32 contraction chunks; x.T tiles arrive via
     DMA-transpose; 8 fp16 matmuls per chunk accumulate into 2 PSUM
     banks -> logitsT [128e, 1024t].
  2. Local top-32 per expert per half (4x max8/max_index/match_replace).
     32 >> max tokens any 1024-token half contributes to a global
     top-160 (Binomial(160, 1/16); measured max 23 on the fixed input).
  3. AllGather the half's candidate values (the first AllGather
     overlaps the second half's matmuls) -> cand [128, 512] laid out
     so that candidate slot order == global token id order for ties.
  4. Global top-160 of cand via 20x max8/max_index/match_replace:
     sorted values g + their slot positions p in cand. max_index's
     first-unused-match rule reproduces lax.top_k tie semantics.
  5. partial_table[e, r] = sum_s (global_id+1)[e,s] * (p[e,r] == own
     slot s) -> each core emits its owned slots of expert_indices;
     host sums the disjoint partials (unshard).
  6. tau[e] = g[:, 159] -> dispatch_mask rows for own tokens =
     (logitsT >= tau); PE-transpose to token-major and write out.
  7. loss = 160 * Ln(1.0) computed on the ACT engine (matches XLA-on-
     neuron's log approximation; exact value is 0 in real arithmetic).
"""

import os

import numpy as np

import concourse.bacc as bacc
import concourse.mybir as mybir
import concourse.tile as tile
from concourse.bass_utils import run_bass_kernel_spmd

N_CORES = 8
T, H, E = 16384, 4096, 128
TSH = T // N_CORES          # tokens per core (2048)
CAP = 160                   # capacity = int(T * 1.25 / E)
HALF = TSH // 2             # 1024-token half, the matmul/topk pipeline unit
K_H = 32                    # candidates per half (4 x 8)
N_IT_H = K_H // 8
K_LOC = 2 * K_H             # candidates per core
N_CAND = N_CORES * K_LOC    # 512
HCH = H // 128              # 32 contraction chunks
NEG = -1.0e30

_CACHE = {}


def _build():
    nc = bacc.Bacc(
        "TRN2",
        target_bir_lowering=False,
        debug=False,
        enable_asserts=True,
        num_devices=N_CORES,
    )
    f32 = mybir.dt.float32
    f16 = mybir.dt.float16

    xh_in = nc.dram_tensor("xh", [TSH, H], f16, kind="ExternalInput")
    xl_in = nc.dram_tensor("xl", [TSH, H], f16, kind="ExternalInput")
    wh_in = nc.dram_tensor("wh", [H, E], f16, kind="ExternalInput")
    wl_in = nc.dram_tensor("wl", [H, E], f16, kind="ExternalInput")
    ident_in = nc.dram_tensor("ident", [128, 128], f32, kind="ExternalInput")
    identh_in = nc.dram_tensor("identh", [128, 128], f16, kind="ExternalInput")
    iota_in = nc.dram_tensor("iota160", [128, CAP], f32, kind="ExternalInput")
    coff_in = nc.dram_tensor("coff", [128, 1], f32, kind="ExternalInput")
    soff_in = nc.dram_tensor("soff", [128, 1], f32, kind="ExternalInput")

    out_table = nc.dram_tensor("out_table", [E, CAP], f32, kind="ExternalOutput")
    out_mask = nc.dram_tensor("out_mask", [TSH, E], f32, kind="ExternalOutput")
    out_loss = nc.dram_tensor("out_loss", [1, 1], f32, kind="ExternalOutput")
    out_cvals = nc.dram_tensor("out_cvals", [E, K_LOC], f32, kind="ExternalOutput")
    out_cids = nc.dram_tensor("out_cids", [E, K_LOC], f32, kind="ExternalOutput")

    with tile.TileContext(nc) as tc:
        with (
            tc.tile_pool(name="const", bufs=1) as cpool,
            tc.tile_pool(name="xt", bufs=4) as xtp,
            tc.tile_pool(name="big", bufs=1) as bpool,
            tc.tile_pool(name="small", bufs=1) as spool,
            tc.tile_pool(name="pst", bufs=1, space="PSUM") as pst,
            tc.tile_pool(name="psm", bufs=2, space="PSUM") as psm,
            tc.tile_pool(name="dram", bufs=1, space="DRAM") as dram,
        ):
            # ---- constants (gpsimd queue; nc.sync is xbar-transpose-only) ----
            wh_sb = cpool.tile([128, HCH, 128], f16)   # (h%128, h//128, e)
            nc.gpsimd.dma_start(wh_sb[:], wh_in.ap().rearrange("(k p) e -> p k e", p=128))
            wl_sb = cpool.tile([128, HCH, 128], f16)
            nc.gpsimd.dma_start(wl_sb[:], wl_in.ap().rearrange("(k p) e -> p k e", p=128))
            ident = cpool.tile([128, 128], f32)
            nc.gpsimd.dma_start(ident[:], ident_in[:])
            identh = cpool.tile([128, 128], f16)
            nc.gpsimd.dma_start(identh[:], identh_in[:])
            iota160 = cpool.tile([128, CAP], f32)
            nc.gpsimd.dma_start(iota160[:], iota_in[:])
            coff = cpool.tile([128, 1], f32)
            nc.gpsimd.dma_start(coff[:], coff_in[:])
            soff = cpool.tile([128, 1], f32)
            nc.gpsimd.dma_start(soff[:], soff_in[:])

            # xl natural for both halves, loaded up-front (no WAR stall at the
            # half boundary; the gpsimd queue is free until the allgathers)
            xlnats = []
            for half in range(2):
                xln = cpool.tile([128, 8, H], f16, name=f"xlnat{half}")
                nc.gpsimd.dma_start(
                    xln[:],
                    xl_in[half * HALF : (half + 1) * HALF, :].rearrange(
                        "(b p) h -> p b h", p=128
                    ),
                )
                xlnats.append(xln)

            logitsT = bpool.tile([128, TSH], f32)      # [e, t_local]
            scr_a = bpool.tile([128, HALF], f32)
            scr_b = bpool.tile([128, HALF], f32)
            l_vals = spool.tile([128, K_LOC], f32)
            l_idx = spool.tile([128, K_LOC], mybir.dt.uint32)
            cc_outs = []

            for half in range(2):
                t0 = half * HALF
                # ---- matmul: logitsT[:, t0:t0+HALF] = W @ xT(half) ----
                # 3 fp16 passes: wh*xh + wh*xl + wl*xh (the wl*xl term is
                # ~1e-7 of the result — far below fp32 rounding).
                # xh.T comes via the DMA-transpose xbar; xl.T is produced on
                # the PE — cheap fp16 transposes that keep the PE busy every
                # HAM window so the clock stays warm.
                xlnat = xlnats[half]
                ps = [
                    psm.tile([128, 512], f32, tag=f"ps{tt}", name=f"ps{half}_{tt}")
                    for tt in range(2)
                ]
                # pass A: the two xh terms — paced by the xbar, independent
                # of the bulk xl loads still streaming in
                for k in range(HCH):
                    xth = xtp.tile([128, HALF], f16, tag="xth")
                    nc.sync.dma_start(
                        xth[:], xh_in[t0 : t0 + HALF, k * 128 : (k + 1) * 128],
                        transpose=True,
                    )
                    for tt in range(2):
                        sl = slice(tt * 512, (tt + 1) * 512)
                        nc.tensor.matmul(ps[tt][:], wh_sb[:, k, :], xth[:, sl],
                                         start=(k == 0), stop=False)
                        nc.tensor.matmul(ps[tt][:], wl_sb[:, k, :], xth[:, sl],
                                         start=False, stop=False)
                # pass B: the wh*xl term — xl.T produced on the PE
                copy_flip = half
                for k in range(HCH):
                    ptl = pst.tile([128, HALF], f16, tag="ptl", bufs=2)
                    for b in range(8):
                        nc.tensor.transpose(
                            ptl[:, b * 128 : (b + 1) * 128],
                            xlnat[:, b, k * 128 : (k + 1) * 128],
                            identh[:],
                        )
                    xtl = xtp.tile([128, HALF], f16, tag="xtl")
                    if copy_flip & 1:
                        nc.vector.tensor_copy(xtl[:], ptl[:])
                    else:
                        nc.scalar.copy(xtl[:], ptl[:])
                    copy_flip += 1
                    for tt in range(2):
                        sl = slice(tt * 512, (tt + 1) * 512)
                        nc.tensor.matmul(ps[tt][:], wh_sb[:, k, :], xtl[:, sl],
                                         start=False, stop=(k == HCH - 1))
                for tt in range(2):
                    nc.scalar.copy(logitsT[:, t0 + tt * 512 : t0 + (tt + 1) * 512],
                                   ps[tt][:])

                # ---- local top-32 for this half ----
                nc.vector.tensor_copy(scr_a[:], logitsT[:, t0 : t0 + HALF])
                cur, nxt = scr_a, scr_b
                for i in range(N_IT_H):
                    sl = slice(K_H * half + 8 * i, K_H * half + 8 * i + 8)
                    nc.vector.max(l_vals[:, sl], cur[:])
                    nc.vector.max_index(l_idx[:, sl], l_vals[:, sl], cur[:])
                    nc.vector.match_replace(nxt[:], l_vals[:, sl], cur[:], NEG)
                    cur, nxt = nxt, cur


            # ---- global ids (+1) of local candidates ----
            ids1 = spool.tile([128, K_LOC], f32)
            nc.vector.tensor_copy(ids1[:], l_idx[:])   # u32 -> f32 cast
            for half in range(2):
                sl = slice(K_H * half, K_H * (half + 1))
                nc.vector.tensor_scalar(ids1[:, sl], ids1[:, sl], coff[:, :],
                                        1.0 + half * HALF,
                                        op0=mybir.AluOpType.add,
                                        op1=mybir.AluOpType.add)
            nc.gpsimd.dma_start(out_cvals[:], l_vals[:])
            nc.gpsimd.dma_start(out_cids[:], ids1[:])

            # ---- allgather all candidate values (one collective) ----
            cc_in = dram.tile([128, K_LOC], f32, name="ccin")
            cc_out = dram.tile([N_CORES * 128, K_LOC], f32,
                               addr_space="Shared", name="ccout")
            nc.gpsimd.dma_start(cc_in[:], l_vals[:])
            nc.gpsimd.collective_compute(
                "AllGather",
                mybir.AluOpType.bypass,
                ins=[cc_in[:].opt()],
                outs=[cc_out[:].opt()],
                replica_groups=[list(range(N_CORES))],
            )
            # slot order (core, half, s) == global token id order, so
            # max_index's tie rule (first unused match) == lax.top_k's
            cand = spool.tile([128, N_CORES, K_LOC], f32)
            nc.gpsimd.dma_start(
                cand[:],
                cc_out[:].rearrange("(c e) s -> e c s", c=N_CORES),
            )
            cand_f = cand[:].rearrange("e c s -> e (c s)")

            # ---- global top-160: sorted values + slot positions ----
            scr_c = spool.tile([128, N_CAND], f32)
            scr_d = spool.tile([128, N_CAND], f32)
            nc.vector.tensor_copy(scr_c[:], cand_f)
            gtop = spool.tile([128, CAP], f32)
            gpos = spool.tile([128, CAP], mybir.dt.uint32)
            cur, nxt = scr_c, scr_d
            for i in range(CAP // 8):
                sl = slice(8 * i, 8 * i + 8)
                nc.vector.max(gtop[:, sl], cur[:])
                nc.vector.max_index(gpos[:, sl], gtop[:, sl], cur[:])
                nc.vector.match_replace(nxt[:], gtop[:, sl], cur[:], NEG)
                cur, nxt = nxt, cur

            # ---- partial expert_indices table ----
            # psh[e, r] = gpos[e, r] - 64*core: own candidate s iff psh == s.
            # Only s < SWIN can be selected (max members/half is 23 << 32);
            # a miss would leave a hole that the host repair fills exactly.
            SWIN = 56
            psh = spool.tile([128, CAP], f32)
            nc.vector.tensor_copy(psh[:], gpos[:])     # u32 -> f32 cast
            nc.vector.tensor_scalar(psh[:], psh[:], soff[:, :], None,
                                    op0=mybir.AluOpType.subtract)
            partial = spool.tile([128, CAP], f32)
            RCH = 40
            eqc = spool.tile([128, RCH, SWIN], f32)
            for r0 in range(0, CAP, RCH):
                psh_b = psh[:, r0 : r0 + RCH].unsqueeze(2).broadcast_to([128, RCH, SWIN])
                iota_b = iota160[:, :SWIN].unsqueeze(1).broadcast_to([128, RCH, SWIN])
                nc.vector.tensor_tensor(eqc[:], psh_b, iota_b, op=mybir.AluOpType.is_equal)
                ids_b = ids1[:, :SWIN].unsqueeze(1).broadcast_to([128, RCH, SWIN])
                nc.vector.tensor_tensor(eqc[:], eqc[:], ids_b, op=mybir.AluOpType.mult)
                nc.vector.tensor_reduce(
                    partial[:, r0 : r0 + RCH], eqc[:],
                    axis=mybir.AxisListType.X, op=mybir.AluOpType.add,
                )
            nc.gpsimd.dma_start(out_table[:], partial[:])

            # ---- dispatch mask: logitsT >= tau = g[:, 159] ----
            # reuse the topk scratch buffers as the [e, t] 0/1 mask halves
            for half in range(2):
                mk = (scr_a, scr_b)[half]
                nc.vector.tensor_scalar(
                    mk[:], logitsT[:, half * HALF : (half + 1) * HALF],
                    gtop[:, CAP - 1 : CAP], None,
                    op0=mybir.AluOpType.is_ge,
                )
            for bt in range(TSH // 128):
                mk = (scr_a, scr_b)[bt // 8]
                ps_t = pst.tile([128, 128], f32, tag="pst", bufs=2)
                nc.tensor.transpose(
                    ps_t[:], mk[:, (bt % 8) * 128 : (bt % 8 + 1) * 128], ident[:]
                )
                mask_t = xtp.tile([128, 128], f32, tag="maskt", bufs=4)
                if bt & 1:
                    nc.scalar.copy(mask_t[:], ps_t[:])
                else:
                    nc.vector.tensor_copy(mask_t[:], ps_t[:])
                nc.gpsimd.dma_start(out_mask[bt * 128 : (bt + 1) * 128, :], mask_t[:])

            # ---- loss = 160 * Ln(1.0) on ACT ----
            onet = spool.tile([1, 1], f32)
            nc.vector.memset(onet[:], 1.0)
            lnt = spool.tile([1, 1], f32)
            nc.scalar.activation(lnt[:], onet[:], mybir.ActivationFunctionType.Ln)
            nc.scalar.mul(lnt[:], lnt[:], float(CAP))
            nc.gpsimd.dma_start(out_loss[:], lnt[:])

    nc.compile()
    return nc


def _get_nc():
    if "nc" not in _CACHE:
        _CACHE["nc"] = _build()
    return _CACHE["nc"]


def _split_f16(a):
    hi = a.astype(np.float16)
    lo = (a - hi.astype(np.float32)).astype(np.float16)
    return hi, lo


def kernel(x, W_gate):
    x = np.asarray(x, dtype=np.float32)
    W_gate = np.asarray(W_gate, dtype=np.float32)
    assert x.shape == (T, H) and W_gate.shape == (E, H)

    xh, xl = _split_f16(x)
    wt = np.ascontiguousarray(W_gate.T)                      # [H, E]
    wh, wl = _split_f16(wt)
    ident = np.eye(128, dtype=np.float32)
    identh = np.eye(128, dtype=np.float16)
    iota160 = np.broadcast_to(
        np.arange(CAP, dtype=np.float32)[None, :], (128, CAP)
    ).copy()

    in_maps = []
    for c in range(N_CORES):
        in_maps.append(
            {
                "xh": np.ascontiguousarray(xh[c * TSH : (c + 1) * TSH]),
                "xl": np.ascontiguousarray(xl[c * TSH : (c + 1) * TSH]),
                "wh": wh,
                "wl": wl,
                "ident": ident,
                "identh": identh,
                "iota160": iota160,
                "coff": np.full((128, 1), c * TSH, dtype=np.float32),
                "soff": np.full((128, 1), c * K_LOC, dtype=np.float32),
            }
        )

    nc = _get_nc()
    trace = bool(int(os.environ.get("KERNEL_TRACE", "0")))
    res = run_bass_kernel_spmd(
        nc, in_maps, core_ids=list(range(N_CORES)), trace=trace
    )
    _CACHE["exec_time_ns"] = res.exec_time_ns
    return _assemble(res.results)


def _assemble(results):
    # unshard: expert_indices slots are disjoint across cores (0 elsewhere)
    table = np.zeros((E, CAP), dtype=np.float64)
    for c in range(N_CORES):
        table += results[c]["out_table"].astype(np.float64)
    if not (table > 0).all():
        # Exact f32 logit tie between two candidates: the device's strict-
        # greater rank gives both a half-integer rank, leaving holes. Rebuild
        # the affected experts' rows from the device-computed candidate
        # (value, id) lists with lax.top_k tie semantics (lower id first).
        cvals = np.stack([results[c]["out_cvals"] for c in range(N_CORES)], 1)  # [E,8,K]
        cids = np.stack([results[c]["out_cids"] for c in range(N_CORES)], 1)
        cvals = cvals.reshape(E, -1)
        cids = cids.reshape(E, -1)  # id+1
        for e in np.unique(np.argwhere(table <= 0)[:, 0]):
            order = np.lexsort((cids[e], -cvals[e].astype(np.float64)))[:CAP]
            table[e] = cids[e][order]
    expert_indices = (table - 1.0).astype(np.int32)

    dispatch_mask = np.concatenate(
        [results[c]["out_mask"] for c in range(N_CORES)], axis=0
    )
    loss = np.float32(results[0]["out_loss"][0, 0])
    return expert_indices, dispatch_mask, loss


# revision 34
# speedup vs baseline: 1.0603x; 1.0603x over previous
"""Arctic expert-choice MoE router on 8 Trainium2 NeuronCores (Bass/Tile).

Problem: x [16384, 4096] f32, W_gate [128, 4096] f32.
  logits = x @ W_gate.T                      [T=16384, E=128]
  expert_indices = top_k(logits.T, 160)      [E, 160]  (per-expert top tokens)
  dispatch_mask[t, e] = 1.0 for selected     [T, E]
  load_balancing_loss = mean(load * log(load/mean_load)) = 160*log(1.0)

Sharding: tokens split across 8 cores (2048 each); W replicated.

Numerics: the matmul runs as 4 fp16 passes (x and W each split into
fp16 hi+lo pairs on the host; products are exact in fp32 PSUM), which
matches fp32 logits to ~1e-6 — enough to reproduce the reference's
top-k ordering exactly (measured min adjacent-gap ~7.5e-7, typical
2.8e-3). fp16 operands let the PE run at 1 cycle/row (vs 4 for fp32)
and let the DMA xbar do the x transpose (2-byte dtypes only).

Per-core device program (SPMD), pipelined over two 1024-token halves:
  1. For each half: 32 contraction chunks; x.T tiles arrive via
     DMA-transpose; 8 fp16 matmuls per chunk accumulate into 2 PSUM
     banks -> logitsT [128e, 1024t].
  2. Local top-32 per expert per half (4x max8/max_index/match_replace).
     32 >> max tokens any 1024-token half contributes to a global
     top-160 (Binomial(160, 1/16); measured max 23 on the fixed input).
  3. AllGather the half's candidate values (the first AllGather
     overlaps the second half's matmuls) -> cand [128, 512] laid out
     so that candidate slot order == global token id order for ties.
  4. Global top-160 of cand via 20x max8/max_index/match_replace:
     sorted values g + their slot positions p in cand. max_index's
     first-unused-match rule reproduces lax.top_k tie semantics.
  5. partial_table[e, r] = sum_s (global_id+1)[e,s] * (p[e,r] == own
     slot s) -> each core emits its owned slots of expert_indices;
     host sums the disjoint partials (unshard).
  6. tau[e] = g[:, 159] -> dispatch_mask rows for own tokens =
     (logitsT >= tau); PE-transpose to token-major and write out.
  7. loss = 160 * Ln(1.0) computed on the ACT engine (matches XLA-on-
     neuron's log approximation; exact value is 0 in real arithmetic).
"""

import os

import numpy as np

import concourse.bacc as bacc
import concourse.mybir as mybir
import concourse.tile as tile
from concourse.bass_utils import run_bass_kernel_spmd

N_CORES = 8
T, H, E = 16384, 4096, 128
TSH = T // N_CORES          # tokens per core (2048)
CAP = 160                   # capacity = int(T * 1.25 / E)
HALF = TSH // 2             # 1024-token half, the matmul/topk pipeline unit
K_H = 32                    # candidates per half (4 x 8)
N_IT_H = K_H // 8
K_LOC = 2 * K_H             # candidates per core
N_CAND = N_CORES * K_LOC    # 512
HCH = H // 128              # 32 contraction chunks
NEG = -1.0e30

_CACHE = {}


def _build():
    nc = bacc.Bacc(
        "TRN2",
        target_bir_lowering=False,
        debug=False,
        enable_asserts=True,
        num_devices=N_CORES,
    )
    f32 = mybir.dt.float32
    f16 = mybir.dt.float16

    xh_in = nc.dram_tensor("xh", [TSH, H], f16, kind="ExternalInput")
    xl_in = nc.dram_tensor("xl", [TSH, H], f16, kind="ExternalInput")
    wh_in = nc.dram_tensor("wh", [H, E], f16, kind="ExternalInput")
    wl_in = nc.dram_tensor("wl", [H, E], f16, kind="ExternalInput")
    ident_in = nc.dram_tensor("ident", [128, 128], f32, kind="ExternalInput")
    identh_in = nc.dram_tensor("identh", [128, 128], f16, kind="ExternalInput")
    iota_in = nc.dram_tensor("iota160", [128, CAP], f32, kind="ExternalInput")
    coff_in = nc.dram_tensor("coff", [128, 1], f32, kind="ExternalInput")
    soff_in = nc.dram_tensor("soff", [128, 1], f32, kind="ExternalInput")

    out_table = nc.dram_tensor("out_table", [E, CAP], f32, kind="ExternalOutput")
    out_mask = nc.dram_tensor("out_mask", [TSH, E], f32, kind="ExternalOutput")
    out_loss = nc.dram_tensor("out_loss", [1, 1], f32, kind="ExternalOutput")
    out_cvals = nc.dram_tensor("out_cvals", [E, K_LOC], f32, kind="ExternalOutput")
    out_cids = nc.dram_tensor("out_cids", [E, K_LOC], f32, kind="ExternalOutput")

    with tile.TileContext(nc) as tc:
        with (
            tc.tile_pool(name="const", bufs=1) as cpool,
            tc.tile_pool(name="xt", bufs=4) as xtp,
            tc.tile_pool(name="big", bufs=1) as bpool,
            tc.tile_pool(name="small", bufs=1) as spool,
            tc.tile_pool(name="pst", bufs=1, space="PSUM") as pst,
            tc.tile_pool(name="psm", bufs=2, space="PSUM") as psm,
            tc.tile_pool(name="dram", bufs=1, space="DRAM") as dram,
        ):
            # ---- constants (gpsimd queue; nc.sync is xbar-transpose-only) ----
            wh_sb = cpool.tile([128, HCH, 128], f16)   # (h%128, h//128, e)
            nc.gpsimd.dma_start(wh_sb[:], wh_in.ap().rearrange("(k p) e -> p k e", p=128))
            wl_sb = cpool.tile([128, HCH, 128], f16)
            nc.gpsimd.dma_start(wl_sb[:], wl_in.ap().rearrange("(k p) e -> p k e", p=128))
            ident = cpool.tile([128, 128], f32)
            nc.gpsimd.dma_start(ident[:], ident_in[:])
            identh = cpool.tile([128, 128], f16)
            nc.gpsimd.dma_start(identh[:], identh_in[:])
            iota160 = cpool.tile([128, CAP], f32)
            nc.gpsimd.dma_start(iota160[:], iota_in[:])
            coff = cpool.tile([128, 1], f32)
            nc.gpsimd.dma_start(coff[:], coff_in[:])
            soff = cpool.tile([128, 1], f32)
            nc.gpsimd.dma_start(soff[:], soff_in[:])

            # xl natural for both halves, loaded up-front (no WAR stall at the
            # half boundary; the gpsimd queue is free until the allgathers)
            xlnats = []
            for half in range(2):
                xln = cpool.tile([128, 8, H], f16, name=f"xlnat{half}")
                nc.gpsimd.dma_start(
                    xln[:],
                    xl_in[half * HALF : (half + 1) * HALF, :].rearrange(
                        "(b p) h -> p b h", p=128
                    ),
                )
                xlnats.append(xln)

            logitsT = bpool.tile([128, TSH], f32)      # [e, t_local]
            scr_a = bpool.tile([128, HALF], f32)
            scr_b = bpool.tile([128, HALF], f32)
            l_vals = spool.tile([128, K_LOC], f32)
            l_idx = spool.tile([128, K_LOC], mybir.dt.uint32)
            cc_outs = []

            for half in range(2):
                t0 = half * HALF
                # ---- matmul: logitsT[:, t0:t0+HALF] = W @ xT(half) ----
                # 3 fp16 passes: wh*xh + wh*xl + wl*xh (the wl*xl term is
                # ~1e-7 of the result — far below fp32 rounding).
                # xh.T comes via the DMA-transpose xbar; xl.T is produced on
                # the PE — cheap fp16 transposes that keep the PE busy every
                # HAM window so the clock stays warm.
                xlnat = xlnats[half]
                ps = [
                    psm.tile([128, 512], f32, tag=f"ps{tt}", name=f"ps{half}_{tt}")
                    for tt in range(2)
                ]
                # pass A: the two xh terms — paced by the xbar, independent
                # of the bulk xl loads still streaming in
                for k in range(HCH):
                    xth = xtp.tile([128, HALF], f16, tag="xth")
                    nc.sync.dma_start(
                        xth[:], xh_in[t0 : t0 + HALF, k * 128 : (k + 1) * 128],
                        transpose=True,
                    )
                    for tt in range(2):
                        sl = slice(tt * 512, (tt + 1) * 512)
                        nc.tensor.matmul(ps[tt][:], wh_sb[:, k, :], xth[:, sl],
                                         start=(k == 0), stop=False)
                        nc.tensor.matmul(ps[tt][:], wl_sb[:, k, :], xth[:, sl],
                                         start=False, stop=False)
                # pass B: the wh*xl term — xl.T produced on the PE
                copy_flip = half
                for k in range(HCH):
                    ptl = pst.tile([128, HALF], f16, tag="ptl", bufs=2)
                    for b in range(8):
                        nc.tensor.transpose(
                            ptl[:, b * 128 : (b + 1) * 128],
                            xlnat[:, b, k * 128 : (k + 1) * 128],
                            identh[:],
                        )
                    xtl = xtp.tile([128, HALF], f16, tag="xtl")
                    if copy_flip & 1:
                        nc.vector.tensor_copy(xtl[:], ptl[:])
                    else:
                        nc.scalar.copy(xtl[:], ptl[:])
                    copy_flip += 1
                    for tt in range(2):
                        sl = slice(tt * 512, (tt + 1) * 512)
                        nc.tensor.matmul(ps[tt][:], wh_sb[:, k, :], xtl[:, sl],
                                         start=False, stop=(k == HCH - 1))
                for tt in range(2):
                    nc.scalar.copy(logitsT[:, t0 + tt * 512 : t0 + (tt + 1) * 512],
                                   ps[tt][:])

                # ---- local top-32 for this half ----
                nc.vector.tensor_copy(scr_a[:], logitsT[:, t0 : t0 + HALF])
                cur, nxt = scr_a, scr_b
                for i in range(N_IT_H):
                    sl = slice(K_H * half + 8 * i, K_H * half + 8 * i + 8)
                    nc.vector.max(l_vals[:, sl], cur[:])
                    nc.vector.max_index(l_idx[:, sl], l_vals[:, sl], cur[:])
                    nc.vector.match_replace(nxt[:], l_vals[:, sl], cur[:], NEG)
                    cur, nxt = nxt, cur

                # ---- allgather this half's candidate values ----
                cc_in = dram.tile([128, K_H], f32, name=f"ccin{half}")
                cc_out = dram.tile([N_CORES * 128, K_H], f32,
                                   addr_space="Shared", name=f"ccout{half}")
                nc.gpsimd.dma_start(cc_in[:], l_vals[:, K_H * half : K_H * (half + 1)])
                nc.gpsimd.collective_compute(
                    "AllGather",
                    mybir.AluOpType.bypass,
                    ins=[cc_in[:].opt()],
                    outs=[cc_out[:].opt()],
                    replica_groups=[list(range(N_CORES))],
                )
                cc_outs.append(cc_out)


            # ---- global ids (+1) of local candidates ----
            ids1 = spool.tile([128, K_LOC], f32)
            nc.vector.tensor_copy(ids1[:], l_idx[:])   # u32 -> f32 cast
            for half in range(2):
                sl = slice(K_H * half, K_H * (half + 1))
                nc.vector.tensor_scalar(ids1[:, sl], ids1[:, sl], coff[:, :],
                                        1.0 + half * HALF,
                                        op0=mybir.AluOpType.add,
                                        op1=mybir.AluOpType.add)
            nc.gpsimd.dma_start(out_cvals[:], l_vals[:])
            nc.gpsimd.dma_start(out_cids[:], ids1[:])

            # ---- collect gathered candidates [e, (core, half, s)] ----
            # slot order == global token id order, so max_index's tie rule
            # (first unused match) == lax.top_k's (lower id first)
            cand = spool.tile([128, N_CORES, 2, K_H], f32)
            for half in range(2):
                nc.gpsimd.dma_start(
                    cand[:, :, half, :],
                    cc_outs[half][:].rearrange("(c e) s -> e c s", c=N_CORES),
                )
            cand_f = cand[:].rearrange("e c h s -> e (c h s)")

            # ---- global top-160: sorted values + slot positions ----
            scr_c = spool.tile([128, N_CAND], f32)
            scr_d = spool.tile([128, N_CAND], f32)
            nc.vector.tensor_copy(scr_c[:], cand_f)
            gtop = spool.tile([128, CAP], f32)
            gpos = spool.tile([128, CAP], mybir.dt.uint32)
            cur, nxt = scr_c, scr_d
            for i in range(CAP // 8):
                sl = slice(8 * i, 8 * i + 8)
                nc.vector.max(gtop[:, sl], cur[:])
                nc.vector.max_index(gpos[:, sl], gtop[:, sl], cur[:])
                nc.vector.match_replace(nxt[:], gtop[:, sl], cur[:], NEG)
                cur, nxt = nxt, cur

            # ---- partial expert_indices table ----
            # psh[e, r] = gpos[e, r] - 64*core: own candidate s iff psh == s.
            # Only s < SWIN can be selected (max members/half is 23 << 32);
            # a miss would leave a hole that the host repair fills exactly.
            SWIN = 56
            psh = spool.tile([128, CAP], f32)
            nc.vector.tensor_copy(psh[:], gpos[:])     # u32 -> f32 cast
            nc.vector.tensor_scalar(psh[:], psh[:], soff[:, :], None,
                                    op0=mybir.AluOpType.subtract)
            partial = spool.tile([128, CAP], f32)
            RCH = 40
            eqc = spool.tile([128, RCH, SWIN], f32)
            for r0 in range(0, CAP, RCH):
                psh_b = psh[:, r0 : r0 + RCH].unsqueeze(2).broadcast_to([128, RCH, SWIN])
                iota_b = iota160[:, :SWIN].unsqueeze(1).broadcast_to([128, RCH, SWIN])
                nc.vector.tensor_tensor(eqc[:], psh_b, iota_b, op=mybir.AluOpType.is_equal)
                ids_b = ids1[:, :SWIN].unsqueeze(1).broadcast_to([128, RCH, SWIN])
                nc.vector.tensor_tensor(eqc[:], eqc[:], ids_b, op=mybir.AluOpType.mult)
                nc.vector.tensor_reduce(
                    partial[:, r0 : r0 + RCH], eqc[:],
                    axis=mybir.AxisListType.X, op=mybir.AluOpType.add,
                )
            nc.gpsimd.dma_start(out_table[:], partial[:])

            # ---- dispatch mask: logitsT >= tau = g[:, 159] ----
            # reuse the topk scratch buffers as the [e, t] 0/1 mask halves
            for half in range(2):
                mk = (scr_a, scr_b)[half]
                nc.vector.tensor_scalar(
                    mk[:], logitsT[:, half * HALF : (half + 1) * HALF],
                    gtop[:, CAP - 1 : CAP], None,
                    op0=mybir.AluOpType.is_ge,
                )
            for bt in range(TSH // 128):
                mk = (scr_a, scr_b)[bt // 8]
                ps_t = pst.tile([128, 128], f32, tag="pst", bufs=2)
                nc.tensor.transpose(
                    ps_t[:], mk[:, (bt % 8) * 128 : (bt % 8 + 1) * 128], ident[:]
                )
                mask_t = xtp.tile([128, 128], f32, tag="maskt", bufs=4)
                if bt & 1:
                    nc.scalar.copy(mask_t[:], ps_t[:])
                else:
                    nc.vector.tensor_copy(mask_t[:], ps_t[:])
                nc.gpsimd.dma_start(out_mask[bt * 128 : (bt + 1) * 128, :], mask_t[:])

            # ---- loss = 160 * Ln(1.0) on ACT ----
            onet = spool.tile([1, 1], f32)
            nc.vector.memset(onet[:], 1.0)
            lnt = spool.tile([1, 1], f32)
            nc.scalar.activation(lnt[:], onet[:], mybir.ActivationFunctionType.Ln)
            nc.scalar.mul(lnt[:], lnt[:], float(CAP))
            nc.gpsimd.dma_start(out_loss[:], lnt[:])

    nc.compile()
    return nc


def _get_nc():
    if "nc" not in _CACHE:
        _CACHE["nc"] = _build()
    return _CACHE["nc"]


def _split_f16(a):
    hi = a.astype(np.float16)
    lo = (a - hi.astype(np.float32)).astype(np.float16)
    return hi, lo


def kernel(x, W_gate):
    x = np.asarray(x, dtype=np.float32)
    W_gate = np.asarray(W_gate, dtype=np.float32)
    assert x.shape == (T, H) and W_gate.shape == (E, H)

    xh, xl = _split_f16(x)
    wt = np.ascontiguousarray(W_gate.T)                      # [H, E]
    wh, wl = _split_f16(wt)
    ident = np.eye(128, dtype=np.float32)
    identh = np.eye(128, dtype=np.float16)
    iota160 = np.broadcast_to(
        np.arange(CAP, dtype=np.float32)[None, :], (128, CAP)
    ).copy()

    in_maps = []
    for c in range(N_CORES):
        in_maps.append(
            {
                "xh": np.ascontiguousarray(xh[c * TSH : (c + 1) * TSH]),
                "xl": np.ascontiguousarray(xl[c * TSH : (c + 1) * TSH]),
                "wh": wh,
                "wl": wl,
                "ident": ident,
                "identh": identh,
                "iota160": iota160,
                "coff": np.full((128, 1), c * TSH, dtype=np.float32),
                "soff": np.full((128, 1), c * K_LOC, dtype=np.float32),
            }
        )

    nc = _get_nc()
    trace = bool(int(os.environ.get("KERNEL_TRACE", "0")))
    res = run_bass_kernel_spmd(
        nc, in_maps, core_ids=list(range(N_CORES)), trace=trace
    )
    _CACHE["exec_time_ns"] = res.exec_time_ns
    return _assemble(res.results)


def _assemble(results):
    # unshard: expert_indices slots are disjoint across cores (0 elsewhere)
    table = np.zeros((E, CAP), dtype=np.float64)
    for c in range(N_CORES):
        table += results[c]["out_table"].astype(np.float64)
    if not (table > 0).all():
        # Exact f32 logit tie between two candidates: the device's strict-
        # greater rank gives both a half-integer rank, leaving holes. Rebuild
        # the affected experts' rows from the device-computed candidate
        # (value, id) lists with lax.top_k tie semantics (lower id first).
        cvals = np.stack([results[c]["out_cvals"] for c in range(N_CORES)], 1)  # [E,8,K]
        cids = np.stack([results[c]["out_cids"] for c in range(N_CORES)], 1)
        cvals = cvals.reshape(E, -1)
        cids = cids.reshape(E, -1)  # id+1
        for e in np.unique(np.argwhere(table <= 0)[:, 0]):
            order = np.lexsort((cids[e], -cvals[e].astype(np.float64)))[:CAP]
            table[e] = cids[e][order]
    expert_indices = (table - 1.0).astype(np.int32)

    dispatch_mask = np.concatenate(
        [results[c]["out_mask"] for c in range(N_CORES)], axis=0
    )
    loss = np.float32(results[0]["out_loss"][0, 0])
    return expert_indices, dispatch_mask, loss
